# revision 23
# baseline (speedup 1.0000x reference)
# GQA attention block on 8 Trainium2 NeuronCores — restructured bf16 edition.
# Sharding: core = (batch b in {0,1}) x (tensor-parallel t in {0..3}).
# Each core: batch row b, 4 query heads {4t..4t+3}, 2 kv heads {2t, 2t+1}.
# W_Q/W_K/W_V split column-wise (per-head), W_O row-wise; the 4 TP partial
# outputs per batch are summed on the host (the "all-reduce").
#
# vs the naive schedule:
#  - softmax exp batched over PSUM bank-pairs (N=1024 per ACT instruction)
#  - softmax denominator moved off the tensor engine: DVE running adds over
#    the ex tiles + one gpsimd partition_all_reduce (PE saves a full second
#    pass over ex), reciprocal via the fast DVE approximation
#  - output projection interleaved per 512-row block so PE never drains
#  - RoPE in bf16 (2x DVE modes), output written as bf16 (halves out DMA)
import math
import sys

sys.path.insert(0, "/opt/trn_rl_repo")

import ml_dtypes
import numpy as np

import concourse.bacc as bacc
import concourse.bass as bass
import concourse.bass_isa as bass_isa
import concourse.mybir as mybir
import concourse.tile as tile
from contextlib import ExitStack

BF = mybir.dt.bfloat16
F32 = mybir.dt.float32
bfnp = ml_dtypes.bfloat16

EMB = 2048
HEADS = 16
G = 2
HD = 128          # head dim
KV = HEADS // G   # 8 kv heads
B = 2
S = 2048
NCORES = 8
TP = 4
HQ = HEADS // TP       # 4 q heads per core
HKV = KV // TP         # 2 kv heads per core
NE = EMB // 128        # 16 contraction chunks
SC4 = S // 512         # 4 s-chunks of 512
SC16 = S // 128        # 16 s-chunks of 128
SCALE = 1.0 / math.sqrt(float(EMB))

_NC = None


def _build_program(loop_n=None):
    nc = bacc.Bacc("TRN2", target_bir_lowering=False, debug=False)

    xT = nc.dram_tensor("xT", (EMB, S), BF, kind="ExternalInput")
    wq = nc.dram_tensor("wq", (EMB, HQ * HD), BF, kind="ExternalInput")
    wk = nc.dram_tensor("wk", (EMB, HKV * HD), BF, kind="ExternalInput")
    wv = nc.dram_tensor("wv", (EMB, HKV * HD), BF, kind="ExternalInput")
    wo = nc.dram_tensor("wo", (HQ * HD, EMB), BF, kind="ExternalInput")
    cosT = nc.dram_tensor("cosT", (HD, S), BF, kind="ExternalInput")
    sinT = nc.dram_tensor("sinT", (HD, S), BF, kind="ExternalInput")
    out = nc.dram_tensor("out", (S, EMB), BF, kind="ExternalOutput")

    with tile.TileContext(nc) as tc, ExitStack() as ctx:
        persist = ctx.enter_context(tc.tile_pool(name="persist", bufs=1))
        # roped Q (jb 0..3) and K (jb 4..5), bf16: [d, jb, sc, s512]
        qk_sb = persist.tile([128, HQ + HKV, SC4, 512], BF)
        # V in [t, d] layout: [t_part, t_chunk, kvl*128+d]
        v_sb = persist.tile([128, SC16, HKV * HD], BF)
        ctx_sb = persist.tile([128, HQ, SC4, 512], BF)   # [d, head, sc, s]
        wo_sb = persist.tile([128, HQ, SC4, 512], BF)    # [d, head, ec, e]
        xt_sb = persist.tile([128, NE, S], BF)
        wqs = persist.tile([128, NE, HQ * HD], BF)
        wks = persist.tile([128, NE, HKV * HD], BF)
        wvs = persist.tile([128, NE, HKV * HD], BF)
        cos_sb = persist.tile([128, SC4, 512], BF)
        sin_sb = persist.tile([128, SC4, 512], BF)

        # batched input loads: few multi-dim DMAs (the SP sequencer pays
        # ~0.6us dispatch per DMA). xT is split so its completion semaphores
        # fire progressively and the first projection can start early; wk/wv
        # chunks are interleaved with it because the first unit's V matmuls
        # consume wv chunk c together with xt chunk c.
        # xT streams on the sync sequencer; everything else dispatches in
        # parallel from the gpsimd SWDGE queue (the SP sequencer pays
        # ~0.6us dispatch per DMA, so splitting the dispatch across two
        # sequencers roughly halves the feed ramp).
        xTr = xT.rearrange("(c p) s -> p c s", p=128)
        wkr = wk.rearrange("(c p) j -> p c j", p=128)
        wvr = wv.rearrange("(c p) j -> p c j", p=128)
        nc.sync.dma_start(out=xt_sb[:, 0:1, :], in_=xTr[:, 0:1, :])
        nc.gpsimd.dma_start(out=wks[:, 0:4, :], in_=wkr[:, 0:4, :])
        nc.gpsimd.dma_start(out=wvs[:, 0:4, :], in_=wvr[:, 0:4, :])
        nc.sync.dma_start(out=xt_sb[:, 1:2, :], in_=xTr[:, 1:2, :])
        nc.gpsimd.dma_start(out=wks[:, 4:16, :], in_=wkr[:, 4:16, :])
        nc.gpsimd.dma_start(out=wvs[:, 4:16, :], in_=wvr[:, 4:16, :])
        for ci in range(1, 8):
            nc.sync.dma_start(
                out=xt_sb[:, 2 * ci:2 * ci + 2, :], in_=xTr[:, 2 * ci:2 * ci + 2, :]
            )
        nc.gpsimd.dma_start(out=wqs, in_=wq.rearrange("(c p) j -> p c j", p=128))
        nc.gpsimd.dma_start(
            out=cos_sb, in_=cosT.rearrange("p (sc s) -> p sc s", s=512))
        nc.gpsimd.dma_start(
            out=sin_sb, in_=sinT.rearrange("p (sc s) -> p sc s", s=512))
        nc.gpsimd.dma_start(
            out=wo_sb, in_=wo.rearrange("(jb p) (ec e) -> p jb ec e", p=128, e=512)
        )

        # PSUM budget (8 banks): pairs 2x2 + accp 2 + oacc 2
        pairs = ctx.enter_context(tc.tile_pool(name="pairs", bufs=2, space="PSUM"))
        accp = ctx.enter_context(tc.tile_pool(name="accp", bufs=2, space="PSUM"))
        oacc = ctx.enter_context(tc.tile_pool(name="oacc", bufs=2, space="PSUM"))
        ropet = ctx.enter_context(tc.tile_pool(name="ropet", bufs=2))
        expool = ctx.enter_context(tc.tile_pool(name="expool", bufs=4))
        dccp = ctx.enter_context(tc.tile_pool(name="dccp", bufs=2))
        darp = ctx.enter_context(tc.tile_pool(name="darp", bufs=2))
        rbp = ctx.enter_context(tc.tile_pool(name="rbp", bufs=1))
        outs = ctx.enter_context(tc.tile_pool(name="outs", bufs=2))

        warm = persist.tile([128, 256], BF)

        def _phases():
            # Pre-warm the ACT "exp" table set while the input DMAs stream:
            # otherwise the first real exp pays the ~2.7us table load in the
            # middle of the kernel.
            nc.vector.memset(warm, 0.0)
            nc.scalar.activation(
                warm[:, 0:16], warm[:, 0:16], mybir.ActivationFunctionType.Exp
            )
            # PE warm-up: dummy matmuls on zeros while the first input DMAs
            # land, so the HAM activity window starts ramping the PE clock
            # before the real projection stream begins (output never read).
            wps = oacc.tile([128, 512], F32, tag="oacc", name="wps")
            for _ in range(40):
                nc.tensor.matmul(
                    wps[:, 0:256], warm[:, 0:128], warm, start=True, stop=True
                )

            # ---------------- Phase 1: projections + RoPE ----------------
            def rope(jb, scp, pt):
                xs = ropet.tile([128, 2, 512], BF, tag="xs")
                if jb in (2, 3):
                    # last Q units: keep the ACT queue clear so attention's
                    # first exp isn't stuck behind these copies
                    nc.vector.tensor_copy(xs, pt)
                else:
                    nc.scalar.copy(xs, pt)
                xw = ropet.tile([128, 2, 512], BF, tag="xw")
                nc.sync.dma_start(out=xw[0:64, :, :], in_=xs[64:128, :, :])
                nc.sync.dma_start(out=xw[64:128, :, :], in_=xs[0:64, :, :])
                csl = slice(2 * scp, 2 * scp + 2)
                nc.vector.tensor_mul(xs, xs, cos_sb[:, csl, :])
                nc.vector.tensor_mul(xw, xw, sin_sb[:, csl, :])
                nc.vector.tensor_add(qk_sb[:, jb, csl, :], xs, xw)

            def jsl_of(jb):
                if jb < HQ:
                    return wqs, slice(jb * 128, (jb + 1) * 128)
                kvl = jb - HQ
                return wks, slice(kvl * 128, (kvl + 1) * 128)

            def do_qk(jb):
                w_sb, jsl = jsl_of(jb)
                for scp in range(2):      # pairs of 512-wide s-chunks
                    pt = pairs.tile([128, 2, 512], F32, tag="pairs")
                    for c in range(NE):
                        lhsT = w_sb[:, c, jsl]
                        for k in range(2):
                            sck = 2 * scp + k
                            nc.tensor.matmul(
                                pt[:, k, :], lhsT,
                                xt_sb[:, c, sck * 512:(sck + 1) * 512],
                                start=(c == 0), stop=(c == NE - 1),
                            )
                    rope(jb, scp, pt)

            def do_qk_v(jb, vsts):
                # chunk-major: the qk unit and its paired V columns consume
                # each xT chunk together, keeping PE ahead of the DMA feed
                # during the initial load window
                w_sb, jsl = jsl_of(jb)
                pt0 = pairs.tile([128, 2, 512], F32, tag="pairs", name=f"pt0_{jb}")
                pt1 = pairs.tile([128, 2, 512], F32, tag="pairs", name=f"pt1_{jb}")
                pvs = []
                for i, st in enumerate(vsts):
                    pool = accp if i < 2 else oacc
                    pvs.append(
                        pool.tile([128, 512], F32,
                                  tag="accp" if i < 2 else "oacc",
                                  name=f"pv_{jb}_{st}")
                    )
                # V matmuls lead the qk matmuls by LEAD chunks: at unit
                # boundaries the first qk matmul waits for the previous
                # unit's rope copies to release the scores psum slots, and
                # the leading V matmuls (own psum pool) fill that latency
                LEAD = 5
                for c in range(NE + LEAD):
                    if c < NE:
                        for i, st in enumerate(vsts):
                            nc.tensor.matmul(
                                pvs[i][:, 0:HKV * HD],
                                xt_sb[:, c, st * 128:(st + 1) * 128],
                                wvs[:, c, :],
                                start=(c == 0), stop=(c == NE - 1),
                            )
                    if c >= LEAD:
                        cq = c - LEAD
                        lhsT = w_sb[:, cq, jsl]
                        for scp, pt in enumerate((pt0, pt1)):
                            for k in range(2):
                                sck = 2 * scp + k
                                nc.tensor.matmul(
                                    pt[:, k, :], lhsT,
                                    xt_sb[:, cq, sck * 512:(sck + 1) * 512],
                                    start=(cq == 0), stop=(cq == NE - 1),
                                )
                rope(jb, 0, pt0)
                rope(jb, 1, pt1)
                for i, st in enumerate(vsts):
                    nc.scalar.copy(v_sb[:, st, :], pvs[i][:, 0:HKV * HD])

            def do_v(sts):
                for st in sts:
                    pv = accp.tile([128, 512], F32, tag="accp")
                    for c in range(NE):
                        nc.tensor.matmul(
                            pv[:, 0:HKV * HD],
                            xt_sb[:, c, st * 128:(st + 1) * 128],
                            wvs[:, c, :],
                            start=(c == 0), stop=(c == NE - 1),
                        )
                    nc.scalar.copy(v_sb[:, st, :], pv[:, 0:HKV * HD])

            # K first (attention h=0 needs it), each early unit dragging 3 V
            # columns chunk-major through the DMA feed window
            do_qk_v(HQ, [0, 1, 2])
            do_qk_v(HQ + 1, [3, 4, 5])
            do_qk_v(0, [6, 7, 8])
            do_qk_v(1, [9, 10, 11])
            do_v([12, 13, 14, 15])
            do_qk(2)
            do_qk(3)

            # ---------- Phase 2+3: attention + output projection ----------
            # Interleaved at head granularity: outproj(sc-1) group so4=j is
            # emitted after attention head (sc, j).  By then the denominator
            # chain (gpsimd all-reduce + recip + mul) for ALL of sc-1's heads
            # has drained, so the outproj matmuls never block the PE queue,
            # and they serve as fill work for the exp-gated attention stream.
            # All PSUM->SBUF copies run on DVE: the ACT engine carries only
            # the exp stream, which paces attention.
            def scores_for(sc_, h_, g):
                kvjb_ = HQ + h_ // 2
                sp = pairs.tile([128, 2, 512], F32, tag="pairs")
                for k in range(2):
                    tcn = 2 * g + k
                    nc.tensor.matmul(
                        sp[:, k, :],
                        qk_sb[:, kvjb_, tcn // 4, (tcn % 4) * 128:(tcn % 4) * 128 + 128],
                        qk_sb[:, h_, sc_, :],
                        start=True, stop=True,
                    )
                return sp

            def attn_head(sc, h, filler, prev_tail, sp0, nxt):
                kvl = h // 2
                cps = accp.tile([128, 512], F32, tag="accp")
                dacc = dccp.tile([128, 2, 512], BF, tag="dacc")

                # scores run one pair ahead of exp/ctx so the static PE
                # stream never blocks on the activation latency; the
                # prefetch crosses head boundaries (sp0 came from the
                # previous head, and this head emits the next head's first
                # pair at g=7) so ACT never idles at a boundary
                sp_next = sp0 if sp0 is not None else scores_for(sc, h, 0)
                sp0_next = None
                ex0 = None
                for g in range(8):        # pairs of 128-wide t-chunks
                    sp = sp_next
                    if g < 7:
                        sp_next = scores_for(sc, h, g + 1)
                    elif nxt is not None:
                        sp0_next = scores_for(nxt[0], nxt[1], 0)
                    ex = expool.tile([128, 2, 512], BF, tag="ex")
                    nc.scalar.activation(
                        ex, sp, mybir.ActivationFunctionType.Exp, scale=SCALE,
                    )
                    for k in range(2):
                        nc.tensor.matmul(
                            cps,
                            v_sb[:, 2 * g + k, kvl * 128:(kvl + 1) * 128],
                            ex[:, k, :],
                            start=(g == 0 and k == 0), stop=(g == 7 and k == 1),
                        )
                    # 2 outproj matmuls woven into each pair: PE fill work
                    # that never blocks on ACT, emitted INSIDE the head so
                    # the next head's scores are not pushed back in the
                    # PE queue by a monolithic outproj block
                    for _ in range(2):
                        next(filler, None)
                    # two running denominator lanes -> one DVE op per pair;
                    # the first add consumes the g=0 and g=1 tiles together
                    if g == 0:
                        ex0 = ex
                    elif g == 1:
                        nc.vector.tensor_add(dacc, ex0, ex)
                    else:
                        nc.vector.tensor_add(dacc, dacc, ex)
                    # the previous head's recip+mul are emitted here, AFTER
                    # this head's first dacc adds: the recip waits on the
                    # 3.5us gpsimd reduce, and the DVE engine queue is strict
                    # FIFO -- emitted at the head end it would block ex-tile
                    # recycling (and thus the exp stream) at the boundary
                    if g == 2 and prev_tail is not None:
                        prev_tail()

                # fold + partition reduce start now (no engine-blocking
                # waits); the reduce runs during the next head's g0-g2
                nc.vector.tensor_add(
                    dacc[:, 0, :], dacc[:, 0, :], dacc[:, 1, :]
                )
                dar = darp.tile([128, 512], F32, tag="dar")
                nc.gpsimd.partition_all_reduce(
                    dar, dacc[:, 0, :], 128, bass_isa.ReduceOp.add
                )

                def tail():
                    rb = rbp.tile([128, 512], F32, tag="rb")
                    nc.vector.reciprocal_approx_fast(rb, dar)
                    nc.vector.tensor_mul(ctx_sb[:, h, sc, :], cps, rb)
                return tail, sp0_next

            # one outproj row-block (128 out rows x full EMB) of sc,
            # as a generator yielding after each matmul
            def outproj_stream(sc, so4):
                ot4 = outs.tile([128, SC4, 512], BF, tag="ot")
                for ec in range(SC4):
                    ops = oacc.tile([128, 512], F32, tag="oacc")
                    for hl in range(HQ):
                        nc.tensor.matmul(
                            ops,
                            ctx_sb[:, hl, sc, so4 * 128:(so4 + 1) * 128],
                            wo_sb[:, hl, ec, :],
                            start=(hl == 0), stop=(hl == HQ - 1),
                        )
                        if hl == HQ - 1:
                            # copy split 3 DVE / 1 ACT keeps both engines
                            # under the PE step time
                            if ec == 3:
                                nc.scalar.copy(ot4[:, ec, :], ops)
                            else:
                                nc.vector.tensor_copy(ot4[:, ec, :], ops)
                            if ec == SC4 - 1:
                                so = sc * 4 + so4
                                nc.sync.dma_start(
                                    out=out[so * 128:(so + 1) * 128, :].rearrange(
                                        "p (ec e) -> p ec e", e=512
                                    ),
                                    in_=ot4,
                                )
                        yield

            def chain_streams(items):
                for sc, so4 in items:
                    yield from outproj_stream(sc, so4)

            # outproj trails attention by one sc block plus one head-step
            # (so group (sc,0)'s hl=3 matmul is always emitted after the
            # deferred mul that writes ctx_sb[sc, h3])
            groups = [(sc, so4) for sc in range(SC4) for so4 in range(4)]
            filler = chain_streams(groups)
            empty = iter(())
            seq = [(sc, h) for sc in range(SC4) for h in range(HQ)]
            prev_tail = None
            sp0 = None
            for si, (sc, h) in enumerate(seq):
                nxt = seq[si + 1] if si + 1 < len(seq) else None
                prev_tail, sp0 = attn_head(
                    sc, h, empty if si < 5 else filler, prev_tail, sp0, nxt
                )
            prev_tail()
            for _ in filler:
                pass

        if loop_n is not None:
            with tc.For_i(0, loop_n, 1):
                _phases()
        else:
            _phases()

    nc.compile()
    return nc


def _get_nc():
    global _NC
    if _NC is None:
        _NC = _build_program()
    return _NC


def _rope_tables():
    half = HD // 2
    inv_freq = 1.0 / (10000.0 ** (np.arange(half, dtype=np.float64) * 2.0 / HD))
    ang = np.arange(S, dtype=np.float64)[:, None] * inv_freq[None, :]  # (S, 64)
    cos = np.concatenate([np.cos(ang), np.cos(ang)], axis=1).T  # (128, S)
    sin = np.concatenate([-np.sin(ang), np.sin(ang)], axis=1).T  # pre-signed
    return (np.ascontiguousarray(cos).astype(bfnp),
            np.ascontiguousarray(sin).astype(bfnp))


def build_in_maps(x, W_Q, W_K, W_V, W_O):
    x = np.asarray(x, dtype=np.float32)
    W_Q = np.asarray(W_Q, dtype=np.float32)
    W_K = np.asarray(W_K, dtype=np.float32)
    W_V = np.asarray(W_V, dtype=np.float32)
    W_O = np.asarray(W_O, dtype=np.float32)
    cos, sin = _rope_tables()
    in_maps = []
    xTb = [np.ascontiguousarray(x[b].T).astype(bfnp) for b in range(B)]
    for b in range(B):
        for t in range(TP):
            qheads = list(range(HQ * t, HQ * t + HQ))
            kvheads = [HKV * t + i for i in range(HKV)]
            idxq = [d * HEADS + h for h in qheads for d in range(HD)]
            idxkv = [d * KV + kv for kv in kvheads for d in range(HD)]
            rows_o = [h * HD + d for h in qheads for d in range(HD)]
            in_maps.append(dict(
                xT=xTb[b],
                wq=np.ascontiguousarray(W_Q[idxq, :].T).astype(bfnp),
                wk=np.ascontiguousarray(W_K[idxkv, :].T).astype(bfnp),
                wv=np.ascontiguousarray(W_V[idxkv, :].T).astype(bfnp),
                wo=np.ascontiguousarray(W_O[:, rows_o].T).astype(bfnp),
                cosT=cos,
                sinT=sin,
            ))
    return in_maps


def emulate_core(m):
    """Numpy emulation of the device math for one core's in_map."""
    xT = np.asarray(m["xT"], np.float32)      # (E, S)
    wq = np.asarray(m["wq"], np.float32)      # (E, 512)
    wk = np.asarray(m["wk"], np.float32)
    wv = np.asarray(m["wv"], np.float32)
    wo = np.asarray(m["wo"], np.float32)      # (512, E)
    cos = np.asarray(m["cosT"], np.float32)   # (128, S)
    sin = np.asarray(m["sinT"], np.float32)

    def bfq(a):
        return a.astype(bfnp).astype(np.float32)

    qT = bfq(wq.T @ xT)                       # (512, S)
    kT = bfq(wk.T @ xT)
    vT = bfq(wv.T @ xT)

    def rope(blkT):  # (128, S)
        xw = np.concatenate([blkT[64:], blkT[:64]], axis=0)
        return bfq(blkT * cos + xw * sin)

    ctxs = []
    for h in range(HQ):
        qh = rope(qT[h * 128:(h + 1) * 128])
        kvl = h // 2
        kh = rope(kT[kvl * 128:(kvl + 1) * 128])
        vh = vT[kvl * 128:(kvl + 1) * 128]
        scoresT = kh.T @ qh * SCALE           # (t, s)
        w = bfq(np.exp(scoresT))
        den = w.sum(axis=0)
        ctxT = bfq((vh @ w) / den[None, :])
        ctxs.append(ctxT)
    ctx = np.concatenate(ctxs, axis=0)        # (512, S)
    return bfq(ctx.T @ wo)


def combine_outs(outs):
    out = np.empty((B, S, EMB), dtype=np.float32)
    for b in range(B):
        acc = np.asarray(outs[TP * b]).astype(np.float32)
        for t in range(1, TP):
            acc = acc + np.asarray(outs[TP * b + t]).astype(np.float32)
        out[b] = acc
    return out


LAST_RESULTS = None


def kernel(x, W_Q, W_K, W_V, W_O):
    global LAST_RESULTS
    from concourse.bass_utils import run_bass_kernel_spmd

    nc = _get_nc()
    in_maps = build_in_maps(x, W_Q, W_K, W_V, W_O)
    res = run_bass_kernel_spmd(nc, in_maps, list(range(NCORES)))
    LAST_RESULTS = res
    outs = [r["out"] for r in res.results]
    return combine_outs(outs)



# revision 29
# speedup vs baseline: 1.0394x; 1.0394x over previous
# GQA attention block on 8 Trainium2 NeuronCores — restructured bf16 edition.
# Sharding: core = (batch b in {0,1}) x (tensor-parallel t in {0..3}).
# Each core: batch row b, 4 query heads {4t..4t+3}, 2 kv heads {2t, 2t+1}.
# W_Q/W_K/W_V split column-wise (per-head), W_O row-wise; the 4 TP partial
# outputs per batch are summed on the host (the "all-reduce").
#
# vs the naive schedule:
#  - softmax exp batched over PSUM bank-pairs (N=1024 per ACT instruction)
#  - softmax denominator moved off the tensor engine: DVE running adds over
#    the ex tiles + one gpsimd partition_all_reduce (PE saves a full second
#    pass over ex), reciprocal via the fast DVE approximation
#  - output projection interleaved per 512-row block so PE never drains
#  - RoPE in bf16 (2x DVE modes), output written as bf16 (halves out DMA)
import math
import sys

sys.path.insert(0, "/opt/trn_rl_repo")

import ml_dtypes
import numpy as np

import concourse.bacc as bacc
import concourse.bass as bass
import concourse.bass_isa as bass_isa
import concourse.mybir as mybir
import concourse.tile as tile
from contextlib import ExitStack

BF = mybir.dt.bfloat16
F32 = mybir.dt.float32
bfnp = ml_dtypes.bfloat16

EMB = 2048
HEADS = 16
G = 2
HD = 128          # head dim
KV = HEADS // G   # 8 kv heads
B = 2
S = 2048
NCORES = 8
TP = 4
HQ = HEADS // TP       # 4 q heads per core
HKV = KV // TP         # 2 kv heads per core
NE = EMB // 128        # 16 contraction chunks
SC4 = S // 512         # 4 s-chunks of 512
SC16 = S // 128        # 16 s-chunks of 128
SCALE = 1.0 / math.sqrt(float(EMB))

_NC = None


def _build_program(loop_n=None):
    nc = bacc.Bacc("TRN2", target_bir_lowering=False, debug=False)

    xT = nc.dram_tensor("xT", (EMB, S), BF, kind="ExternalInput")
    wq = nc.dram_tensor("wq", (EMB, HQ * HD), BF, kind="ExternalInput")
    wk = nc.dram_tensor("wk", (EMB, HKV * HD), BF, kind="ExternalInput")
    wv = nc.dram_tensor("wv", (EMB, HKV * HD), BF, kind="ExternalInput")
    wo = nc.dram_tensor("wo", (HQ * HD, EMB), BF, kind="ExternalInput")
    cosT = nc.dram_tensor("cosT", (HD, S), BF, kind="ExternalInput")
    sinT = nc.dram_tensor("sinT", (HD, S), BF, kind="ExternalInput")
    out = nc.dram_tensor("out", (S, EMB), BF, kind="ExternalOutput")

    with tile.TileContext(nc) as tc, ExitStack() as ctx:
        persist = ctx.enter_context(tc.tile_pool(name="persist", bufs=1))
        # roped Q (jb 0..3) and K (jb 4..5), bf16: [d, jb, sc, s512]
        qk_sb = persist.tile([128, HQ + HKV, SC4, 512], BF)
        # V in [t, d] layout: [t_part, t_chunk, kvl*128+d]
        v_sb = persist.tile([128, SC16, HKV * HD], BF)
        ctx_sb = persist.tile([128, HQ, SC4, 512], BF)   # [d, head, sc, s]
        wo_sb = persist.tile([128, HQ, SC4, 512], BF)    # [d, head, ec, e]
        xt_sb = persist.tile([128, NE, S], BF)
        wqs = persist.tile([128, NE, HQ * HD], BF)
        wks = persist.tile([128, NE, HKV * HD], BF)
        wvs = persist.tile([128, NE, HKV * HD], BF)
        cos_sb = persist.tile([128, SC4, 512], BF)
        sin_sb = persist.tile([128, SC4, 512], BF)

        # batched input loads: few multi-dim DMAs (the SP sequencer pays
        # ~0.6us dispatch per DMA). xT is split so its completion semaphores
        # fire progressively and the first projection can start early; wk/wv
        # chunks are interleaved with it because the first unit's V matmuls
        # consume wv chunk c together with xt chunk c.
        # The first unit consumes xt, wk AND wv chunk-by-chunk, so all three
        # stream interleaved in consumption order, fine-grained (2-chunk
        # granules) so completion semaphores fire progressively.  wq/cos/
        # sin/wo follow -- they are consumed much later.
        xTr = xT.rearrange("(c p) s -> p c s", p=128)
        wkr = wk.rearrange("(c p) j -> p c j", p=128)
        wvr = wv.rearrange("(c p) j -> p c j", p=128)
        for ci in range(8):
            cs = slice(2 * ci, 2 * ci + 2)
            nc.sync.dma_start(out=xt_sb[:, cs, :], in_=xTr[:, cs, :])
            nc.sync.dma_start(out=wks[:, cs, :], in_=wkr[:, cs, :])
            nc.sync.dma_start(out=wvs[:, cs, :], in_=wvr[:, cs, :])
        nc.sync.dma_start(out=wqs, in_=wq.rearrange("(c p) j -> p c j", p=128))
        nc.sync.dma_start(out=cos_sb, in_=cosT.rearrange("p (sc s) -> p sc s", s=512))
        nc.sync.dma_start(out=sin_sb, in_=sinT.rearrange("p (sc s) -> p sc s", s=512))
        nc.sync.dma_start(
            out=wo_sb, in_=wo.rearrange("(jb p) (ec e) -> p jb ec e", p=128, e=512)
        )

        # PSUM budget (8 banks): pairs 2x2 + accp 2 + oacc 2
        pairs = ctx.enter_context(tc.tile_pool(name="pairs", bufs=2, space="PSUM"))
        accp = ctx.enter_context(tc.tile_pool(name="accp", bufs=2, space="PSUM"))
        oacc = ctx.enter_context(tc.tile_pool(name="oacc", bufs=2, space="PSUM"))
        # expool slots are shared with the phase-1 rope tiles (same shape,
        # disjoint lifetime) via a single tag
        expool = ctx.enter_context(tc.tile_pool(name="expool", bufs=6))
        dccp = ctx.enter_context(tc.tile_pool(name="dccp", bufs=2))
        darp = ctx.enter_context(tc.tile_pool(name="darp", bufs=2))
        rbp = ctx.enter_context(tc.tile_pool(name="rbp", bufs=1))
        outs = ctx.enter_context(tc.tile_pool(name="outs", bufs=3))

        warm = persist.tile([128, 256], BF)

        def _phases():
            # Pre-warm the ACT "exp" table set while the input DMAs stream:
            # otherwise the first real exp pays the ~2.7us table load in the
            # middle of the kernel.
            nc.vector.memset(warm, 0.0)
            nc.scalar.activation(
                warm[:, 0:16], warm[:, 0:16], mybir.ActivationFunctionType.Exp
            )
            # PE warm-up: dummy matmuls on zeros while the first input DMAs
            # land, so the HAM activity window starts ramping the PE clock
            # before the real projection stream begins (output never read).
            wps = oacc.tile([128, 512], F32, tag="oacc", name="wps")
            for _ in range(40):
                nc.tensor.matmul(
                    wps[:, 0:256], warm[:, 0:128], warm, start=True, stop=True
                )

            # ---------------- Phase 1: projections + RoPE ----------------
            def rope(jb, scp, pt):
                xs = expool.tile([128, 2, 512], BF, tag="ex")
                if jb in (2, 3):
                    # last Q units: keep the ACT queue clear so attention's
                    # first exp isn't stuck behind these copies
                    nc.vector.tensor_copy(xs, pt)
                else:
                    nc.scalar.copy(xs, pt)
                xw = expool.tile([128, 2, 512], BF, tag="ex")
                nc.sync.dma_start(out=xw[0:64, :, :], in_=xs[64:128, :, :])
                nc.sync.dma_start(out=xw[64:128, :, :], in_=xs[0:64, :, :])
                csl = slice(2 * scp, 2 * scp + 2)
                nc.vector.tensor_mul(xs, xs, cos_sb[:, csl, :])
                nc.vector.tensor_mul(xw, xw, sin_sb[:, csl, :])
                nc.vector.tensor_add(qk_sb[:, jb, csl, :], xs, xw)

            def jsl_of(jb):
                if jb < HQ:
                    return wqs, slice(jb * 128, (jb + 1) * 128)
                kvl = jb - HQ
                return wks, slice(kvl * 128, (kvl + 1) * 128)

            def do_qk(jb):
                w_sb, jsl = jsl_of(jb)
                for scp in range(2):      # pairs of 512-wide s-chunks
                    pt = pairs.tile([128, 2, 512], F32, tag="pairs")
                    for c in range(NE):
                        lhsT = w_sb[:, c, jsl]
                        for k in range(2):
                            sck = 2 * scp + k
                            nc.tensor.matmul(
                                pt[:, k, :], lhsT,
                                xt_sb[:, c, sck * 512:(sck + 1) * 512],
                                start=(c == 0), stop=(c == NE - 1),
                            )
                    rope(jb, scp, pt)

            def do_qk_v(jb, vsts):
                # chunk-major: the qk unit and its paired V columns consume
                # each xT chunk together, keeping PE ahead of the DMA feed
                # during the initial load window
                w_sb, jsl = jsl_of(jb)
                pt0 = pairs.tile([128, 2, 512], F32, tag="pairs", name=f"pt0_{jb}")
                pt1 = pairs.tile([128, 2, 512], F32, tag="pairs", name=f"pt1_{jb}")
                pvs = []
                for i, st in enumerate(vsts):
                    pool = accp if i < 2 else oacc
                    pvs.append(
                        pool.tile([128, 512], F32,
                                  tag="accp" if i < 2 else "oacc",
                                  name=f"pv_{jb}_{st}")
                    )
                # V matmuls lead the qk matmuls by LEAD chunks: at unit
                # boundaries the first qk matmul waits for the previous
                # unit's rope copies to release the scores psum slots, and
                # the leading V matmuls (own psum pool) fill that latency
                LEAD = 5
                for c in range(NE + LEAD):
                    if c < NE:
                        for i, st in enumerate(vsts):
                            nc.tensor.matmul(
                                pvs[i][:, 0:HKV * HD],
                                xt_sb[:, c, st * 128:(st + 1) * 128],
                                wvs[:, c, :],
                                start=(c == 0), stop=(c == NE - 1),
                            )
                    if c >= LEAD:
                        cq = c - LEAD
                        lhsT = w_sb[:, cq, jsl]
                        for scp, pt in enumerate((pt0, pt1)):
                            for k in range(2):
                                sck = 2 * scp + k
                                nc.tensor.matmul(
                                    pt[:, k, :], lhsT,
                                    xt_sb[:, cq, sck * 512:(sck + 1) * 512],
                                    start=(cq == 0), stop=(cq == NE - 1),
                                )
                rope(jb, 0, pt0)
                rope(jb, 1, pt1)
                for i, st in enumerate(vsts):
                    nc.scalar.copy(v_sb[:, st, :], pvs[i][:, 0:HKV * HD])

            def do_v(sts):
                for st in sts:
                    pv = accp.tile([128, 512], F32, tag="accp")
                    for c in range(NE):
                        nc.tensor.matmul(
                            pv[:, 0:HKV * HD],
                            xt_sb[:, c, st * 128:(st + 1) * 128],
                            wvs[:, c, :],
                            start=(c == 0), stop=(c == NE - 1),
                        )
                    nc.scalar.copy(v_sb[:, st, :], pv[:, 0:HKV * HD])

            # K first (attention h=0 needs it), each early unit dragging 3 V
            # columns chunk-major through the DMA feed window
            do_qk_v(HQ, [0, 1, 2])
            do_qk_v(HQ + 1, [3, 4, 5])
            do_qk_v(0, [6, 7, 8])
            do_qk_v(1, [9, 10, 11])
            do_v([12, 13, 14, 15])
            do_qk(2)
            do_qk(3)

            # ---------- Phase 2+3: attention + output projection ----------
            # Interleaved at head granularity: outproj(sc-1) group so4=j is
            # emitted after attention head (sc, j).  By then the denominator
            # chain (gpsimd all-reduce + recip + mul) for ALL of sc-1's heads
            # has drained, so the outproj matmuls never block the PE queue,
            # and they serve as fill work for the exp-gated attention stream.
            # All PSUM->SBUF copies run on DVE: the ACT engine carries only
            # the exp stream, which paces attention.
            def scores_for(sc_, h_, g):
                kvjb_ = HQ + h_ // 2
                sp = pairs.tile([128, 2, 512], F32, tag="pairs")
                for k in range(2):
                    tcn = 2 * g + k
                    nc.tensor.matmul(
                        sp[:, k, :],
                        qk_sb[:, kvjb_, tcn // 4, (tcn % 4) * 128:(tcn % 4) * 128 + 128],
                        qk_sb[:, h_, sc_, :],
                        start=True, stop=True,
                    )
                return sp

            def attn_head(sc, h, filler, prev_tail, sp0, nxt):
                kvl = h // 2
                cps = accp.tile([128, 512], F32, tag="accp")
                dacc = dccp.tile([128, 2, 512], BF, tag="dacc")

                # scores run one pair ahead of exp/ctx so the static PE
                # stream never blocks on the activation latency; the
                # prefetch crosses head boundaries (sp0 came from the
                # previous head, and this head emits the next head's first
                # pair at g=7) so ACT never idles at a boundary
                sp_next = sp0 if sp0 is not None else scores_for(sc, h, 0)
                sp0_next = None
                ex0 = None
                for g in range(8):        # pairs of 128-wide t-chunks
                    sp = sp_next
                    if g < 7:
                        sp_next = scores_for(sc, h, g + 1)
                    elif nxt is not None:
                        sp0_next = scores_for(nxt[0], nxt[1], 0)
                    ex = expool.tile([128, 2, 512], BF, tag="ex")
                    nc.scalar.activation(
                        ex, sp, mybir.ActivationFunctionType.Exp, scale=SCALE,
                    )
                    for k in range(2):
                        nc.tensor.matmul(
                            cps,
                            v_sb[:, 2 * g + k, kvl * 128:(kvl + 1) * 128],
                            ex[:, k, :],
                            start=(g == 0 and k == 0), stop=(g == 7 and k == 1),
                        )
                    # two running denominator lanes -> one DVE op per pair;
                    # the first add consumes the g=0 and g=1 tiles together.
                    # Emitted BEFORE the weave so the ex-releasing adds
                    # always lead the DVE FIFO within a pair.
                    if g == 0:
                        ex0 = ex
                    elif g == 1:
                        nc.vector.tensor_add(dacc, ex0, ex)
                    else:
                        nc.vector.tensor_add(dacc, dacc, ex)
                    # 2 outproj matmuls woven into each pair: PE fill work
                    # that never blocks on ACT, emitted INSIDE the head so
                    # the next head's scores are not pushed back in the
                    # PE queue by a monolithic outproj block
                    for _ in range(2):
                        next(filler, None)
                    # the previous head's recip+mul are emitted here (g=5:
                    # its gpsimd reduce, started at the head boundary, is
                    # certainly done, so the recip never blocks the strict-
                    # FIFO DVE queue and ex-tile recycling stays on pace)
                    if g == 5 and prev_tail is not None:
                        prev_tail()

                # fold + partition reduce start now (no engine-blocking
                # waits); the reduce runs during the next head's g0-g2
                nc.vector.tensor_add(
                    dacc[:, 0, :], dacc[:, 0, :], dacc[:, 1, :]
                )
                dar = darp.tile([128, 512], F32, tag="dar")
                nc.gpsimd.partition_all_reduce(
                    dar, dacc[:, 0, :], 128, bass_isa.ReduceOp.add
                )

                def tail():
                    rb = rbp.tile([128, 512], F32, tag="rb")
                    nc.vector.reciprocal_approx_fast(rb, dar)
                    nc.vector.tensor_mul(ctx_sb[:, h, sc, :], cps, rb)
                return tail, sp0_next

            # one outproj row-block (128 out rows x full EMB) of sc,
            # as a generator yielding after each matmul
            def outproj_stream(sc, so4):
                ot4 = outs.tile([128, SC4, 512], BF, tag="ot")
                for ec in range(SC4):
                    ops = oacc.tile([128, 512], F32, tag="oacc")
                    for hl in range(HQ):
                        nc.tensor.matmul(
                            ops,
                            ctx_sb[:, hl, sc, so4 * 128:(so4 + 1) * 128],
                            wo_sb[:, hl, ec, :],
                            start=(hl == 0), stop=(hl == HQ - 1),
                        )
                        if hl == HQ - 1:
                            # copy split 3 DVE / 1 ACT keeps both engines
                            # under the PE step time
                            if ec == 3:
                                nc.scalar.copy(ot4[:, ec, :], ops)
                            else:
                                nc.vector.tensor_copy(ot4[:, ec, :], ops)
                            if ec == SC4 - 1:
                                so = sc * 4 + so4
                                nc.sync.dma_start(
                                    out=out[so * 128:(so + 1) * 128, :].rearrange(
                                        "p (ec e) -> p ec e", e=512
                                    ),
                                    in_=ot4,
                                )
                        yield

            def chain_streams(items):
                for sc, so4 in items:
                    yield from outproj_stream(sc, so4)

            # outproj trails attention by one sc block plus one head-step
            # (so group (sc,0)'s hl=3 matmul is always emitted after the
            # deferred mul that writes ctx_sb[sc, h3])
            groups = [(sc, so4) for sc in range(SC4) for so4 in range(4)]
            filler = chain_streams(groups)
            empty = iter(())
            seq = [(sc, h) for sc in range(SC4) for h in range(HQ)]
            prev_tail = None
            sp0 = None
            for si, (sc, h) in enumerate(seq):
                nxt = seq[si + 1] if si + 1 < len(seq) else None
                prev_tail, sp0 = attn_head(
                    sc, h, empty if si < 5 else filler, prev_tail, sp0, nxt
                )
            prev_tail()
            for _ in filler:
                pass

        if loop_n is not None:
            with tc.For_i(0, loop_n, 1):
                _phases()
        else:
            _phases()

    nc.compile()
    return nc


def _get_nc():
    global _NC
    if _NC is None:
        _NC = _build_program()
    return _NC


def _rope_tables():
    half = HD // 2
    inv_freq = 1.0 / (10000.0 ** (np.arange(half, dtype=np.float64) * 2.0 / HD))
    ang = np.arange(S, dtype=np.float64)[:, None] * inv_freq[None, :]  # (S, 64)
    cos = np.concatenate([np.cos(ang), np.cos(ang)], axis=1).T  # (128, S)
    sin = np.concatenate([-np.sin(ang), np.sin(ang)], axis=1).T  # pre-signed
    return (np.ascontiguousarray(cos).astype(bfnp),
            np.ascontiguousarray(sin).astype(bfnp))


def build_in_maps(x, W_Q, W_K, W_V, W_O):
    x = np.asarray(x, dtype=np.float32)
    W_Q = np.asarray(W_Q, dtype=np.float32)
    W_K = np.asarray(W_K, dtype=np.float32)
    W_V = np.asarray(W_V, dtype=np.float32)
    W_O = np.asarray(W_O, dtype=np.float32)
    cos, sin = _rope_tables()
    in_maps = []
    xTb = [np.ascontiguousarray(x[b].T).astype(bfnp) for b in range(B)]
    for b in range(B):
        for t in range(TP):
            qheads = list(range(HQ * t, HQ * t + HQ))
            kvheads = [HKV * t + i for i in range(HKV)]
            idxq = [d * HEADS + h for h in qheads for d in range(HD)]
            idxkv = [d * KV + kv for kv in kvheads for d in range(HD)]
            rows_o = [h * HD + d for h in qheads for d in range(HD)]
            in_maps.append(dict(
                xT=xTb[b],
                wq=np.ascontiguousarray(W_Q[idxq, :].T).astype(bfnp),
                wk=np.ascontiguousarray(W_K[idxkv, :].T).astype(bfnp),
                wv=np.ascontiguousarray(W_V[idxkv, :].T).astype(bfnp),
                wo=np.ascontiguousarray(W_O[:, rows_o].T).astype(bfnp),
                cosT=cos,
                sinT=sin,
            ))
    return in_maps


def emulate_core(m):
    """Numpy emulation of the device math for one core's in_map."""
    xT = np.asarray(m["xT"], np.float32)      # (E, S)
    wq = np.asarray(m["wq"], np.float32)      # (E, 512)
    wk = np.asarray(m["wk"], np.float32)
    wv = np.asarray(m["wv"], np.float32)
    wo = np.asarray(m["wo"], np.float32)      # (512, E)
    cos = np.asarray(m["cosT"], np.float32)   # (128, S)
    sin = np.asarray(m["sinT"], np.float32)

    def bfq(a):
        return a.astype(bfnp).astype(np.float32)

    qT = bfq(wq.T @ xT)                       # (512, S)
    kT = bfq(wk.T @ xT)
    vT = bfq(wv.T @ xT)

    def rope(blkT):  # (128, S)
        xw = np.concatenate([blkT[64:], blkT[:64]], axis=0)
        return bfq(blkT * cos + xw * sin)

    ctxs = []
    for h in range(HQ):
        qh = rope(qT[h * 128:(h + 1) * 128])
        kvl = h // 2
        kh = rope(kT[kvl * 128:(kvl + 1) * 128])
        vh = vT[kvl * 128:(kvl + 1) * 128]
        scoresT = kh.T @ qh * SCALE           # (t, s)
        w = bfq(np.exp(scoresT))
        den = w.sum(axis=0)
        ctxT = bfq((vh @ w) / den[None, :])
        ctxs.append(ctxT)
    ctx = np.concatenate(ctxs, axis=0)        # (512, S)
    return bfq(ctx.T @ wo)


def combine_outs(outs):
    out = np.empty((B, S, EMB), dtype=np.float32)
    for b in range(B):
        acc = np.asarray(outs[TP * b]).astype(np.float32)
        for t in range(1, TP):
            acc = acc + np.asarray(outs[TP * b + t]).astype(np.float32)
        out[b] = acc
    return out


LAST_RESULTS = None


def kernel(x, W_Q, W_K, W_V, W_O):
    global LAST_RESULTS
    from concourse.bass_utils import run_bass_kernel_spmd

    nc = _get_nc()
    in_maps = build_in_maps(x, W_Q, W_K, W_V, W_O)
    res = run_bass_kernel_spmd(nc, in_maps, list(range(NCORES)))
    LAST_RESULTS = res
    outs = [r["out"] for r in res.results]
    return combine_outs(outs)



# revision 30
# speedup vs baseline: 1.0576x; 1.0175x over previous
# GQA attention block on 8 Trainium2 NeuronCores — restructured bf16 edition.
# Sharding: core = (batch b in {0,1}) x (tensor-parallel t in {0..3}).
# Each core: batch row b, 4 query heads {4t..4t+3}, 2 kv heads {2t, 2t+1}.
# W_Q/W_K/W_V split column-wise (per-head), W_O row-wise; the 4 TP partial
# outputs per batch are summed on the host (the "all-reduce").
#
# vs the naive schedule:
#  - softmax exp batched over PSUM bank-pairs (N=1024 per ACT instruction)
#  - softmax denominator moved off the tensor engine: DVE running adds over
#    the ex tiles + one gpsimd partition_all_reduce (PE saves a full second
#    pass over ex), reciprocal via the fast DVE approximation
#  - output projection interleaved per 512-row block so PE never drains
#  - RoPE in bf16 (2x DVE modes), output written as bf16 (halves out DMA)
import math
import sys

sys.path.insert(0, "/opt/trn_rl_repo")

import ml_dtypes
import numpy as np

import concourse.bacc as bacc
import concourse.bass as bass
import concourse.bass_isa as bass_isa
import concourse.mybir as mybir
import concourse.tile as tile
from contextlib import ExitStack

BF = mybir.dt.bfloat16
F32 = mybir.dt.float32
bfnp = ml_dtypes.bfloat16

EMB = 2048
HEADS = 16
G = 2
HD = 128          # head dim
KV = HEADS // G   # 8 kv heads
B = 2
S = 2048
NCORES = 8
TP = 4
HQ = HEADS // TP       # 4 q heads per core
HKV = KV // TP         # 2 kv heads per core
NE = EMB // 128        # 16 contraction chunks
SC4 = S // 512         # 4 s-chunks of 512
SC16 = S // 128        # 16 s-chunks of 128
SCALE = 1.0 / math.sqrt(float(EMB))

_NC = None


def _build_program(loop_n=None):
    nc = bacc.Bacc("TRN2", target_bir_lowering=False, debug=False)

    xT = nc.dram_tensor("xT", (EMB, S), BF, kind="ExternalInput")
    wq = nc.dram_tensor("wq", (EMB, HQ * HD), BF, kind="ExternalInput")
    wk = nc.dram_tensor("wk", (EMB, HKV * HD), BF, kind="ExternalInput")
    wv = nc.dram_tensor("wv", (EMB, HKV * HD), BF, kind="ExternalInput")
    wo = nc.dram_tensor("wo", (HQ * HD, EMB), BF, kind="ExternalInput")
    cosT = nc.dram_tensor("cosT", (HD, S), BF, kind="ExternalInput")
    sinT = nc.dram_tensor("sinT", (HD, S), BF, kind="ExternalInput")
    out = nc.dram_tensor("out", (S, EMB), BF, kind="ExternalOutput")

    with tile.TileContext(nc) as tc, ExitStack() as ctx:
        persist = ctx.enter_context(tc.tile_pool(name="persist", bufs=1))
        # roped Q (jb 0..3) and K (jb 4..5), bf16: [d, jb, sc, s512]
        qk_sb = persist.tile([128, HQ + HKV, SC4, 512], BF)
        # V in [t, d] layout: [t_part, t_chunk, kvl*128+d]
        v_sb = persist.tile([128, SC16, HKV * HD], BF)
        ctx_sb = persist.tile([128, HQ, SC4, 512], BF)   # [d, head, sc, s]
        wo_sb = persist.tile([128, HQ, SC4, 512], BF)    # [d, head, ec, e]
        xt_sb = persist.tile([128, NE, S], BF)
        wqs = persist.tile([128, NE, HQ * HD], BF)
        wks = persist.tile([128, NE, HKV * HD], BF)
        wvs = persist.tile([128, NE, HKV * HD], BF)
        cos_sb = persist.tile([128, SC4, 512], BF)
        sin_sb = persist.tile([128, SC4, 512], BF)

        # batched input loads: few multi-dim DMAs (the SP sequencer pays
        # ~0.6us dispatch per DMA). xT is split so its completion semaphores
        # fire progressively and the first projection can start early; wk/wv
        # chunks are interleaved with it because the first unit's V matmuls
        # consume wv chunk c together with xt chunk c.
        # The first unit consumes xt, wk AND wv chunk-by-chunk, so all three
        # stream interleaved in consumption order, fine-grained (2-chunk
        # granules) so completion semaphores fire progressively.  wq/cos/
        # sin/wo follow -- they are consumed much later.
        xTr = xT.rearrange("(c p) s -> p c s", p=128)
        wkr = wk.rearrange("(c p) j -> p c j", p=128)
        wvr = wv.rearrange("(c p) j -> p c j", p=128)
        for ci in range(8):
            cs = slice(2 * ci, 2 * ci + 2)
            nc.sync.dma_start(out=xt_sb[:, cs, :], in_=xTr[:, cs, :])
            nc.sync.dma_start(out=wks[:, cs, :], in_=wkr[:, cs, :])
            nc.sync.dma_start(out=wvs[:, cs, :], in_=wvr[:, cs, :])
        nc.sync.dma_start(out=wqs, in_=wq.rearrange("(c p) j -> p c j", p=128))
        nc.sync.dma_start(out=cos_sb, in_=cosT.rearrange("p (sc s) -> p sc s", s=512))
        nc.sync.dma_start(out=sin_sb, in_=sinT.rearrange("p (sc s) -> p sc s", s=512))
        nc.sync.dma_start(
            out=wo_sb, in_=wo.rearrange("(jb p) (ec e) -> p jb ec e", p=128, e=512)
        )

        # PSUM budget (8 banks): pairs 2x2 + accp 2 + oacc 2
        pairs = ctx.enter_context(tc.tile_pool(name="pairs", bufs=2, space="PSUM"))
        accp = ctx.enter_context(tc.tile_pool(name="accp", bufs=2, space="PSUM"))
        oacc = ctx.enter_context(tc.tile_pool(name="oacc", bufs=2, space="PSUM"))
        # expool slots are shared with the phase-1 rope tiles (same shape,
        # disjoint lifetime) via a single tag
        expool = ctx.enter_context(tc.tile_pool(name="expool", bufs=6))
        dccp = ctx.enter_context(tc.tile_pool(name="dccp", bufs=2))
        darp = ctx.enter_context(tc.tile_pool(name="darp", bufs=2))
        rbp = ctx.enter_context(tc.tile_pool(name="rbp", bufs=1))
        outs = ctx.enter_context(tc.tile_pool(name="outs", bufs=3))

        warm = persist.tile([128, 256], BF)

        def _phases():
            # Pre-warm the ACT "exp" table set while the input DMAs stream:
            # otherwise the first real exp pays the ~2.7us table load in the
            # middle of the kernel.
            nc.vector.memset(warm, 0.0)
            nc.scalar.activation(
                warm[:, 0:16], warm[:, 0:16], mybir.ActivationFunctionType.Exp
            )
            # PE warm-up: dummy matmuls on zeros while the first input DMAs
            # land, so the HAM activity window starts ramping the PE clock
            # before the real projection stream begins (output never read).
            wps = oacc.tile([128, 512], F32, tag="oacc", name="wps")
            for _ in range(40):
                nc.tensor.matmul(
                    wps[:, 0:256], warm[:, 0:128], warm, start=True, stop=True
                )

            # ---------------- Phase 1: projections + RoPE ----------------
            def rope(jb, scp, pt):
                xs = expool.tile([128, 2, 512], BF, tag="ex")
                if jb in (2, 3):
                    # last Q units: keep the ACT queue clear so attention's
                    # first exp isn't stuck behind these copies
                    nc.vector.tensor_copy(xs, pt)
                else:
                    nc.scalar.copy(xs, pt)
                xw = expool.tile([128, 2, 512], BF, tag="ex")
                nc.sync.dma_start(out=xw[0:64, :, :], in_=xs[64:128, :, :])
                nc.sync.dma_start(out=xw[64:128, :, :], in_=xs[0:64, :, :])
                csl = slice(2 * scp, 2 * scp + 2)
                nc.vector.tensor_mul(xs, xs, cos_sb[:, csl, :])
                nc.vector.tensor_mul(xw, xw, sin_sb[:, csl, :])
                nc.vector.tensor_add(qk_sb[:, jb, csl, :], xs, xw)

            def jsl_of(jb):
                if jb < HQ:
                    return wqs, slice(jb * 128, (jb + 1) * 128)
                kvl = jb - HQ
                return wks, slice(kvl * 128, (kvl + 1) * 128)

            def do_qk(jb):
                w_sb, jsl = jsl_of(jb)
                for scp in range(2):      # pairs of 512-wide s-chunks
                    pt = pairs.tile([128, 2, 512], F32, tag="pairs")
                    for c in range(NE):
                        lhsT = w_sb[:, c, jsl]
                        for k in range(2):
                            sck = 2 * scp + k
                            nc.tensor.matmul(
                                pt[:, k, :], lhsT,
                                xt_sb[:, c, sck * 512:(sck + 1) * 512],
                                start=(c == 0), stop=(c == NE - 1),
                            )
                    rope(jb, scp, pt)

            def do_qk_v(jb, vsts):
                # chunk-major: the qk unit and its paired V columns consume
                # each xT chunk together, keeping PE ahead of the DMA feed
                # during the initial load window
                w_sb, jsl = jsl_of(jb)
                pt0 = pairs.tile([128, 2, 512], F32, tag="pairs", name=f"pt0_{jb}")
                pt1 = pairs.tile([128, 2, 512], F32, tag="pairs", name=f"pt1_{jb}")
                pvs = []
                for i, st in enumerate(vsts):
                    pool = accp if i < 2 else oacc
                    pvs.append(
                        pool.tile([128, 512], F32,
                                  tag="accp" if i < 2 else "oacc",
                                  name=f"pv_{jb}_{st}")
                    )
                # V matmuls lead the qk matmuls by LEAD chunks: at unit
                # boundaries the first qk matmul waits for the previous
                # unit's rope copies to release the scores psum slots, and
                # the leading V matmuls (own psum pool) fill that latency
                LEAD = 5
                for c in range(NE + LEAD):
                    if c < NE:
                        for i, st in enumerate(vsts):
                            nc.tensor.matmul(
                                pvs[i][:, 0:HKV * HD],
                                xt_sb[:, c, st * 128:(st + 1) * 128],
                                wvs[:, c, :],
                                start=(c == 0), stop=(c == NE - 1),
                            )
                    if c >= LEAD:
                        cq = c - LEAD
                        lhsT = w_sb[:, cq, jsl]
                        for scp, pt in enumerate((pt0, pt1)):
                            for k in range(2):
                                sck = 2 * scp + k
                                nc.tensor.matmul(
                                    pt[:, k, :], lhsT,
                                    xt_sb[:, cq, sck * 512:(sck + 1) * 512],
                                    start=(cq == 0), stop=(cq == NE - 1),
                                )
                rope(jb, 0, pt0)
                rope(jb, 1, pt1)
                for i, st in enumerate(vsts):
                    nc.scalar.copy(v_sb[:, st, :], pvs[i][:, 0:HKV * HD])

            def do_v(sts):
                for st in sts:
                    pv = accp.tile([128, 512], F32, tag="accp")
                    for c in range(NE):
                        nc.tensor.matmul(
                            pv[:, 0:HKV * HD],
                            xt_sb[:, c, st * 128:(st + 1) * 128],
                            wvs[:, c, :],
                            start=(c == 0), stop=(c == NE - 1),
                        )
                    nc.scalar.copy(v_sb[:, st, :], pv[:, 0:HKV * HD])

            # K first (attention h=0 needs it), each early unit dragging 3 V
            # columns chunk-major through the DMA feed window
            do_qk_v(HQ, [0, 1, 2])
            do_qk_v(HQ + 1, [3, 4, 5])
            do_qk_v(0, [6, 7, 8])
            do_qk_v(1, [9, 10, 11])
            do_v([12, 13, 14, 15])
            do_qk(2)
            do_qk(3)

            # ---------- Phase 2+3: attention + output projection ----------
            # Interleaved at head granularity: outproj(sc-1) group so4=j is
            # emitted after attention head (sc, j).  By then the denominator
            # chain (gpsimd all-reduce + recip + mul) for ALL of sc-1's heads
            # has drained, so the outproj matmuls never block the PE queue,
            # and they serve as fill work for the exp-gated attention stream.
            # All PSUM->SBUF copies run on DVE: the ACT engine carries only
            # the exp stream, which paces attention.
            def scores_for(sc_, h_, g):
                kvjb_ = HQ + h_ // 2
                sp = pairs.tile([128, 2, 512], F32, tag="pairs")
                for k in range(2):
                    tcn = 2 * g + k
                    nc.tensor.matmul(
                        sp[:, k, :],
                        qk_sb[:, kvjb_, tcn // 4, (tcn % 4) * 128:(tcn % 4) * 128 + 128],
                        qk_sb[:, h_, sc_, :],
                        start=True, stop=True,
                    )
                return sp

            def attn_head(sc, h, filler, prev_tail, sp0, nxt):
                kvl = h // 2
                cps = accp.tile([128, 512], F32, tag="accp")
                dacc = dccp.tile([128, 2, 512], BF, tag="dacc")

                # scores run one pair ahead of exp/ctx so the static PE
                # stream never blocks on the activation latency; the
                # prefetch crosses head boundaries (sp0 came from the
                # previous head, and this head emits the next head's first
                # pair at g=7) so ACT never idles at a boundary
                sp_next = sp0 if sp0 is not None else scores_for(sc, h, 0)
                sp0_next = None
                ex0 = None
                for g in range(8):        # pairs of 128-wide t-chunks
                    sp = sp_next
                    if g < 7:
                        sp_next = scores_for(sc, h, g + 1)
                    elif nxt is not None:
                        sp0_next = scores_for(nxt[0], nxt[1], 0)
                    ex = expool.tile([128, 2, 512], BF, tag="ex")
                    nc.scalar.activation(
                        ex, sp, mybir.ActivationFunctionType.Exp, scale=SCALE,
                    )
                    for k in range(2):
                        nc.tensor.matmul(
                            cps,
                            v_sb[:, 2 * g + k, kvl * 128:(kvl + 1) * 128],
                            ex[:, k, :],
                            start=(g == 0 and k == 0), stop=(g == 7 and k == 1),
                        )
                    # two running denominator lanes -> one DVE op per pair;
                    # the first add consumes the g=0 and g=1 tiles together.
                    # Emitted BEFORE the weave so the ex-releasing adds
                    # always lead the DVE FIFO within a pair.
                    if g == 0:
                        ex0 = ex
                    elif g == 1:
                        nc.vector.tensor_add(dacc, ex0, ex)
                    else:
                        nc.vector.tensor_add(dacc, dacc, ex)
                    # 2 outproj matmuls woven into each pair: PE fill work
                    # that never blocks on ACT, emitted INSIDE the head so
                    # the next head's scores are not pushed back in the
                    # PE queue by a monolithic outproj block
                    for _ in range(2):
                        next(filler, None)
                    # the previous head's recip+mul are emitted here (g=5:
                    # its gpsimd reduce, started at the head boundary, is
                    # certainly done, so the recip never blocks the strict-
                    # FIFO DVE queue and ex-tile recycling stays on pace)
                    if g == 5 and prev_tail is not None:
                        prev_tail()

                # fold + partition reduce start now (no engine-blocking
                # waits); the reduce runs during the next head's g0-g2
                nc.vector.tensor_add(
                    dacc[:, 0, :], dacc[:, 0, :], dacc[:, 1, :]
                )
                dar = darp.tile([128, 512], F32, tag="dar")
                nc.gpsimd.partition_all_reduce(
                    dar, dacc[:, 0, :], 128, bass_isa.ReduceOp.add
                )

                def tail():
                    rb = rbp.tile([128, 512], F32, tag="rb")
                    nc.vector.reciprocal_approx_fast(rb, dar)
                    nc.vector.tensor_mul(ctx_sb[:, h, sc, :], cps, rb)
                return tail, sp0_next

            # one outproj row-block (128 out rows x full EMB) of sc,
            # as a generator yielding after each matmul
            def outproj_stream(sc, so4):
                ot4 = outs.tile([128, SC4, 512], BF, tag="ot")
                for ec in range(SC4):
                    ops = oacc.tile([128, 512], F32, tag="oacc")
                    for hl in range(HQ):
                        nc.tensor.matmul(
                            ops,
                            ctx_sb[:, hl, sc, so4 * 128:(so4 + 1) * 128],
                            wo_sb[:, hl, ec, :],
                            start=(hl == 0), stop=(hl == HQ - 1),
                        )
                        if hl == HQ - 1:
                            # copy split 2 DVE / 2 ACT: alternating engines
                            # decouples consecutive psum-bank releases, so a
                            # lagging DVE FIFO can't stall the ec+2 matmuls
                            if ec % 2 == 1:
                                nc.scalar.copy(ot4[:, ec, :], ops)
                            else:
                                nc.vector.tensor_copy(ot4[:, ec, :], ops)
                            if ec == SC4 - 1:
                                so = sc * 4 + so4
                                nc.sync.dma_start(
                                    out=out[so * 128:(so + 1) * 128, :].rearrange(
                                        "p (ec e) -> p ec e", e=512
                                    ),
                                    in_=ot4,
                                )
                        yield

            def chain_streams(items):
                for sc, so4 in items:
                    yield from outproj_stream(sc, so4)

            # outproj trails attention by one sc block plus one head-step
            # (so group (sc,0)'s hl=3 matmul is always emitted after the
            # deferred mul that writes ctx_sb[sc, h3])
            groups = [(sc, so4) for sc in range(SC4) for so4 in range(4)]
            filler = chain_streams(groups)
            empty = iter(())
            seq = [(sc, h) for sc in range(SC4) for h in range(HQ)]
            prev_tail = None
            sp0 = None
            for si, (sc, h) in enumerate(seq):
                nxt = seq[si + 1] if si + 1 < len(seq) else None
                prev_tail, sp0 = attn_head(
                    sc, h, empty if si < 5 else filler, prev_tail, sp0, nxt
                )
            prev_tail()
            for _ in filler:
                pass

        if loop_n is not None:
            with tc.For_i(0, loop_n, 1):
                _phases()
        else:
            _phases()

    nc.compile()
    return nc


def _get_nc():
    global _NC
    if _NC is None:
        _NC = _build_program()
    return _NC


def _rope_tables():
    half = HD // 2
    inv_freq = 1.0 / (10000.0 ** (np.arange(half, dtype=np.float64) * 2.0 / HD))
    ang = np.arange(S, dtype=np.float64)[:, None] * inv_freq[None, :]  # (S, 64)
    cos = np.concatenate([np.cos(ang), np.cos(ang)], axis=1).T  # (128, S)
    sin = np.concatenate([-np.sin(ang), np.sin(ang)], axis=1).T  # pre-signed
    return (np.ascontiguousarray(cos).astype(bfnp),
            np.ascontiguousarray(sin).astype(bfnp))


def build_in_maps(x, W_Q, W_K, W_V, W_O):
    x = np.asarray(x, dtype=np.float32)
    W_Q = np.asarray(W_Q, dtype=np.float32)
    W_K = np.asarray(W_K, dtype=np.float32)
    W_V = np.asarray(W_V, dtype=np.float32)
    W_O = np.asarray(W_O, dtype=np.float32)
    cos, sin = _rope_tables()
    in_maps = []
    xTb = [np.ascontiguousarray(x[b].T).astype(bfnp) for b in range(B)]
    for b in range(B):
        for t in range(TP):
            qheads = list(range(HQ * t, HQ * t + HQ))
            kvheads = [HKV * t + i for i in range(HKV)]
            idxq = [d * HEADS + h for h in qheads for d in range(HD)]
            idxkv = [d * KV + kv for kv in kvheads for d in range(HD)]
            rows_o = [h * HD + d for h in qheads for d in range(HD)]
            in_maps.append(dict(
                xT=xTb[b],
                wq=np.ascontiguousarray(W_Q[idxq, :].T).astype(bfnp),
                wk=np.ascontiguousarray(W_K[idxkv, :].T).astype(bfnp),
                wv=np.ascontiguousarray(W_V[idxkv, :].T).astype(bfnp),
                wo=np.ascontiguousarray(W_O[:, rows_o].T).astype(bfnp),
                cosT=cos,
                sinT=sin,
            ))
    return in_maps


def emulate_core(m):
    """Numpy emulation of the device math for one core's in_map."""
    xT = np.asarray(m["xT"], np.float32)      # (E, S)
    wq = np.asarray(m["wq"], np.float32)      # (E, 512)
    wk = np.asarray(m["wk"], np.float32)
    wv = np.asarray(m["wv"], np.float32)
    wo = np.asarray(m["wo"], np.float32)      # (512, E)
    cos = np.asarray(m["cosT"], np.float32)   # (128, S)
    sin = np.asarray(m["sinT"], np.float32)

    def bfq(a):
        return a.astype(bfnp).astype(np.float32)

    qT = bfq(wq.T @ xT)                       # (512, S)
    kT = bfq(wk.T @ xT)
    vT = bfq(wv.T @ xT)

    def rope(blkT):  # (128, S)
        xw = np.concatenate([blkT[64:], blkT[:64]], axis=0)
        return bfq(blkT * cos + xw * sin)

    ctxs = []
    for h in range(HQ):
        qh = rope(qT[h * 128:(h + 1) * 128])
        kvl = h // 2
        kh = rope(kT[kvl * 128:(kvl + 1) * 128])
        vh = vT[kvl * 128:(kvl + 1) * 128]
        scoresT = kh.T @ qh * SCALE           # (t, s)
        w = bfq(np.exp(scoresT))
        den = w.sum(axis=0)
        ctxT = bfq((vh @ w) / den[None, :])
        ctxs.append(ctxT)
    ctx = np.concatenate(ctxs, axis=0)        # (512, S)
    return bfq(ctx.T @ wo)


def combine_outs(outs):
    out = np.empty((B, S, EMB), dtype=np.float32)
    for b in range(B):
        acc = np.asarray(outs[TP * b]).astype(np.float32)
        for t in range(1, TP):
            acc = acc + np.asarray(outs[TP * b + t]).astype(np.float32)
        out[b] = acc
    return out


LAST_RESULTS = None


def kernel(x, W_Q, W_K, W_V, W_O):
    global LAST_RESULTS
    from concourse.bass_utils import run_bass_kernel_spmd

    nc = _get_nc()
    in_maps = build_in_maps(x, W_Q, W_K, W_V, W_O)
    res = run_bass_kernel_spmd(nc, in_maps, list(range(NCORES)))
    LAST_RESULTS = res
    outs = [r["out"] for r in res.results]
    return combine_outs(outs)



# revision 35
# speedup vs baseline: 1.0661x; 1.0081x over previous
# GQA attention block on 8 Trainium2 NeuronCores — restructured bf16 edition.
# Sharding: core = (batch b in {0,1}) x (tensor-parallel t in {0..3}).
# Each core: batch row b, 4 query heads {4t..4t+3}, 2 kv heads {2t, 2t+1}.
# W_Q/W_K/W_V split column-wise (per-head), W_O row-wise; the 4 TP partial
# outputs per batch are summed on the host (the "all-reduce").
#
# vs the naive schedule:
#  - softmax exp batched over PSUM bank-pairs (N=1024 per ACT instruction)
#  - softmax denominator moved off the tensor engine: DVE running adds over
#    the ex tiles + one gpsimd partition_all_reduce (PE saves a full second
#    pass over ex), reciprocal via the fast DVE approximation
#  - output projection interleaved per 512-row block so PE never drains
#  - RoPE in bf16 (2x DVE modes), output written as bf16 (halves out DMA)
import math
import sys

sys.path.insert(0, "/opt/trn_rl_repo")

import ml_dtypes
import numpy as np

import concourse.bacc as bacc
import concourse.bass as bass
import concourse.bass_isa as bass_isa
import concourse.mybir as mybir
import concourse.tile as tile
from contextlib import ExitStack

BF = mybir.dt.bfloat16
F32 = mybir.dt.float32
bfnp = ml_dtypes.bfloat16

EMB = 2048
HEADS = 16
G = 2
HD = 128          # head dim
KV = HEADS // G   # 8 kv heads
B = 2
S = 2048
NCORES = 8
TP = 4
HQ = HEADS // TP       # 4 q heads per core
HKV = KV // TP         # 2 kv heads per core
NE = EMB // 128        # 16 contraction chunks
SC4 = S // 512         # 4 s-chunks of 512
SC16 = S // 128        # 16 s-chunks of 128
SCALE = 1.0 / math.sqrt(float(EMB))

_NC = None


def _build_program(loop_n=None):
    nc = bacc.Bacc("TRN2", target_bir_lowering=False, debug=False)

    xT = nc.dram_tensor("xT", (EMB, S), BF, kind="ExternalInput")
    wq = nc.dram_tensor("wq", (EMB, HQ * HD), BF, kind="ExternalInput")
    wk = nc.dram_tensor("wk", (EMB, HKV * HD), BF, kind="ExternalInput")
    wv = nc.dram_tensor("wv", (EMB, HKV * HD), BF, kind="ExternalInput")
    wo = nc.dram_tensor("wo", (HQ * HD, EMB), BF, kind="ExternalInput")
    cosT = nc.dram_tensor("cosT", (HD, S), BF, kind="ExternalInput")
    sinT = nc.dram_tensor("sinT", (HD, S), BF, kind="ExternalInput")
    out = nc.dram_tensor("out", (S, EMB), BF, kind="ExternalOutput")

    with tile.TileContext(nc) as tc, ExitStack() as ctx:
        persist = ctx.enter_context(tc.tile_pool(name="persist", bufs=1))
        # roped Q (jb 0..3) and K (jb 4..5), bf16: [d, jb, sc, s512]
        qk_sb = persist.tile([128, HQ + HKV, SC4, 512], BF)
        # V in [t, d] layout: [t_part, t_chunk, kvl*128+d]
        v_sb = persist.tile([128, SC16, HKV * HD], BF)
        ctx_sb = persist.tile([128, HQ, SC4, 512], BF)   # [d, head, sc, s]
        wo_sb = persist.tile([128, HQ, SC4, 512], BF)    # [d, head, ec, e]
        xt_sb = persist.tile([128, NE, S], BF)
        wqs = persist.tile([128, NE, HQ * HD], BF)
        wks = persist.tile([128, NE, HKV * HD], BF)
        wvs = persist.tile([128, NE, HKV * HD], BF)
        cos_sb = persist.tile([128, SC4, 512], BF)
        sin_sb = persist.tile([128, SC4, 512], BF)

        # batched input loads: few multi-dim DMAs (the SP sequencer pays
        # ~0.6us dispatch per DMA). xT is split so its completion semaphores
        # fire progressively and the first projection can start early; wk/wv
        # chunks are interleaved with it because the first unit's V matmuls
        # consume wv chunk c together with xt chunk c.
        # The first unit consumes xt, wk AND wv chunk-by-chunk, so all three
        # stream interleaved in consumption order, fine-grained (2-chunk
        # granules) so completion semaphores fire progressively.  wq/cos/
        # sin/wo follow -- they are consumed much later.
        xTr = xT.rearrange("(c p) s -> p c s", p=128)
        wkr = wk.rearrange("(c p) j -> p c j", p=128)
        wvr = wv.rearrange("(c p) j -> p c j", p=128)
        for ci in range(8):
            cs = slice(2 * ci, 2 * ci + 2)
            nc.sync.dma_start(out=xt_sb[:, cs, :], in_=xTr[:, cs, :])
            nc.sync.dma_start(out=wks[:, cs, :], in_=wkr[:, cs, :])
            nc.sync.dma_start(out=wvs[:, cs, :], in_=wvr[:, cs, :])
        nc.sync.dma_start(out=wqs, in_=wq.rearrange("(c p) j -> p c j", p=128))
        nc.sync.dma_start(out=cos_sb, in_=cosT.rearrange("p (sc s) -> p sc s", s=512))
        nc.sync.dma_start(out=sin_sb, in_=sinT.rearrange("p (sc s) -> p sc s", s=512))
        nc.sync.dma_start(
            out=wo_sb, in_=wo.rearrange("(jb p) (ec e) -> p jb ec e", p=128, e=512)
        )

        # PSUM budget (8 banks): pairs 2x2 + accp 2 + oacc 2
        pairs = ctx.enter_context(tc.tile_pool(name="pairs", bufs=2, space="PSUM"))
        accp = ctx.enter_context(tc.tile_pool(name="accp", bufs=2, space="PSUM"))
        oacc = ctx.enter_context(tc.tile_pool(name="oacc", bufs=2, space="PSUM"))
        # expool slots are shared with the phase-1 rope tiles (same shape,
        # disjoint lifetime) via a single tag
        expool = ctx.enter_context(tc.tile_pool(name="expool", bufs=6))
        dccp = ctx.enter_context(tc.tile_pool(name="dccp", bufs=2))
        darp = ctx.enter_context(tc.tile_pool(name="darp", bufs=2))
        rbp = ctx.enter_context(tc.tile_pool(name="rbp", bufs=1))
        outs = ctx.enter_context(tc.tile_pool(name="outs", bufs=3))

        warm = persist.tile([128, 256], BF)

        def _phases():
            # Pre-warm the ACT "exp" table set while the input DMAs stream:
            # otherwise the first real exp pays the ~2.7us table load in the
            # middle of the kernel.
            nc.vector.memset(warm, 0.0)
            nc.scalar.activation(
                warm[:, 0:16], warm[:, 0:16], mybir.ActivationFunctionType.Exp
            )
            # PE warm-up: dummy matmuls on zeros while the first input DMAs
            # land, so the HAM activity window starts ramping the PE clock
            # before the real projection stream begins (output never read).
            wps = oacc.tile([128, 512], F32, tag="oacc", name="wps")
            for _ in range(40):
                nc.tensor.matmul(
                    wps[:, 0:256], warm[:, 0:128], warm, start=True, stop=True
                )

            # ---------------- Phase 1: projections + RoPE ----------------
            def rope(jb, scp, pt):
                xs = expool.tile([128, 2, 512], BF, tag="ex")
                if jb in (2, 3):
                    # last Q units: keep the ACT queue clear so attention's
                    # first exp isn't stuck behind these copies
                    nc.vector.tensor_copy(xs, pt)
                else:
                    nc.scalar.copy(xs, pt)
                xw = expool.tile([128, 2, 512], BF, tag="ex")
                nc.sync.dma_start(out=xw[0:64, :, :], in_=xs[64:128, :, :])
                nc.sync.dma_start(out=xw[64:128, :, :], in_=xs[0:64, :, :])
                csl = slice(2 * scp, 2 * scp + 2)
                nc.vector.tensor_mul(xs, xs, cos_sb[:, csl, :])
                nc.vector.tensor_mul(xw, xw, sin_sb[:, csl, :])
                nc.vector.tensor_add(qk_sb[:, jb, csl, :], xs, xw)

            def jsl_of(jb):
                if jb < HQ:
                    return wqs, slice(jb * 128, (jb + 1) * 128)
                kvl = jb - HQ
                return wks, slice(kvl * 128, (kvl + 1) * 128)

            def do_qk(jb):
                w_sb, jsl = jsl_of(jb)
                for scp in range(2):      # pairs of 512-wide s-chunks
                    pt = pairs.tile([128, 2, 512], F32, tag="pairs")
                    for c in range(NE):
                        lhsT = w_sb[:, c, jsl]
                        for k in range(2):
                            sck = 2 * scp + k
                            nc.tensor.matmul(
                                pt[:, k, :], lhsT,
                                xt_sb[:, c, sck * 512:(sck + 1) * 512],
                                start=(c == 0), stop=(c == NE - 1),
                            )
                    rope(jb, scp, pt)

            def do_qk_v(jb, vsts):
                # chunk-major: the qk unit and its paired V columns consume
                # each xT chunk together, keeping PE ahead of the DMA feed
                # during the initial load window
                w_sb, jsl = jsl_of(jb)
                pt0 = pairs.tile([128, 2, 512], F32, tag="pairs", name=f"pt0_{jb}")
                pt1 = pairs.tile([128, 2, 512], F32, tag="pairs", name=f"pt1_{jb}")
                pvs = []
                for i, st in enumerate(vsts):
                    pool = accp if i < 2 else oacc
                    pvs.append(
                        pool.tile([128, 512], F32,
                                  tag="accp" if i < 2 else "oacc",
                                  name=f"pv_{jb}_{st}")
                    )
                # V matmuls lead the qk matmuls by LEAD chunks: at unit
                # boundaries the first qk matmul waits for the previous
                # unit's rope copies to release the scores psum slots, and
                # the leading V matmuls (own psum pool) fill that latency
                LEAD = 5
                for c in range(NE + LEAD):
                    if c < NE:
                        for i, st in enumerate(vsts):
                            nc.tensor.matmul(
                                pvs[i][:, 0:HKV * HD],
                                xt_sb[:, c, st * 128:(st + 1) * 128],
                                wvs[:, c, :],
                                start=(c == 0), stop=(c == NE - 1),
                            )
                    if c >= LEAD:
                        cq = c - LEAD
                        lhsT = w_sb[:, cq, jsl]
                        for scp, pt in enumerate((pt0, pt1)):
                            for k in range(2):
                                sck = 2 * scp + k
                                nc.tensor.matmul(
                                    pt[:, k, :], lhsT,
                                    xt_sb[:, cq, sck * 512:(sck + 1) * 512],
                                    start=(cq == 0), stop=(cq == NE - 1),
                                )
                rope(jb, 0, pt0)
                rope(jb, 1, pt1)
                for i, st in enumerate(vsts):
                    nc.scalar.copy(v_sb[:, st, :], pvs[i][:, 0:HKV * HD])

            def do_v(sts):
                for st in sts:
                    pv = accp.tile([128, 512], F32, tag="accp")
                    for c in range(NE):
                        nc.tensor.matmul(
                            pv[:, 0:HKV * HD],
                            xt_sb[:, c, st * 128:(st + 1) * 128],
                            wvs[:, c, :],
                            start=(c == 0), stop=(c == NE - 1),
                        )
                    nc.scalar.copy(v_sb[:, st, :], pv[:, 0:HKV * HD])

            # K first (attention h=0 needs it), each early unit dragging 3 V
            # columns chunk-major through the DMA feed window
            do_qk_v(HQ, [0, 1, 2])
            do_qk_v(HQ + 1, [3, 4, 5])
            do_qk_v(0, [6, 7, 8])
            do_qk_v(1, [9, 10, 11])
            do_v([12, 13, 14, 15])
            do_qk(2)
            do_qk(3)

            # ---------- Phase 2+3: attention + output projection ----------
            # Interleaved at head granularity: outproj(sc-1) group so4=j is
            # emitted after attention head (sc, j).  By then the denominator
            # chain (gpsimd all-reduce + recip + mul) for ALL of sc-1's heads
            # has drained, so the outproj matmuls never block the PE queue,
            # and they serve as fill work for the exp-gated attention stream.
            # All PSUM->SBUF copies run on DVE: the ACT engine carries only
            # the exp stream, which paces attention.
            def scores_for(sc_, h_, g):
                kvjb_ = HQ + h_ // 2
                sp = pairs.tile([128, 2, 512], F32, tag="pairs")
                for k in range(2):
                    tcn = 2 * g + k
                    nc.tensor.matmul(
                        sp[:, k, :],
                        qk_sb[:, kvjb_, tcn // 4, (tcn % 4) * 128:(tcn % 4) * 128 + 128],
                        qk_sb[:, h_, sc_, :],
                        start=True, stop=True,
                    )
                return sp

            def attn_head(sc, h, filler, prev_tail, sp0, nxt):
                kvl = h // 2
                cps = accp.tile([128, 512], F32, tag="accp")
                dacc = dccp.tile([128, 2, 512], BF, tag="dacc")

                # scores run one pair ahead of exp/ctx so the static PE
                # stream never blocks on the activation latency; the
                # prefetch crosses head boundaries (sp0 came from the
                # previous head, and this head emits the next head's first
                # pair at g=7) so ACT never idles at a boundary
                sp_next = sp0 if sp0 is not None else scores_for(sc, h, 0)
                sp0_next = None
                ex0 = None
                for g in range(8):        # pairs of 128-wide t-chunks
                    sp = sp_next
                    if g < 7:
                        sp_next = scores_for(sc, h, g + 1)
                    elif nxt is not None:
                        sp0_next = scores_for(nxt[0], nxt[1], 0)
                    ex = expool.tile([128, 2, 512], BF, tag="ex")
                    nc.scalar.activation(
                        ex, sp, mybir.ActivationFunctionType.Exp, scale=SCALE,
                    )
                    for k in range(2):
                        nc.tensor.matmul(
                            cps,
                            v_sb[:, 2 * g + k, kvl * 128:(kvl + 1) * 128],
                            ex[:, k, :],
                            start=(g == 0 and k == 0), stop=(g == 7 and k == 1),
                        )
                    # two running denominator lanes -> one DVE op per pair;
                    # the first add consumes the g=0 and g=1 tiles together.
                    # Emitted BEFORE the weave so the ex-releasing adds
                    # always lead the DVE FIFO within a pair.
                    if g == 0:
                        ex0 = ex
                    elif g == 1:
                        nc.vector.tensor_add(dacc, ex0, ex)
                    else:
                        nc.vector.tensor_add(dacc, dacc, ex)
                    # 2 outproj matmuls woven into each pair: PE fill work
                    # that never blocks on ACT, emitted INSIDE the head so
                    # the next head's scores are not pushed back in the
                    # PE queue by a monolithic outproj block
                    for _ in range(2):
                        next(filler, None)
                    # the previous head's recip+mul are emitted here (g=6:
                    # its gpsimd reduce, started at the head boundary, is
                    # certainly done, so the recip never blocks the strict-
                    # FIFO DVE queue and ex-tile recycling stays on pace;
                    # g=6 also puts it behind the g5-weave CAST that
                    # releases the outproj psum bank)
                    if g == 6 and prev_tail is not None:
                        prev_tail()

                # fold + partition reduce start now (no engine-blocking
                # waits); the reduce runs during the next head's g0-g2
                if nxt is not None:
                    nc.vector.tensor_add(
                        dacc[:, 0, :], dacc[:, 0, :], dacc[:, 1, :]
                    )
                    dar = darp.tile([128, 512], F32, tag="dar")
                    nc.gpsimd.partition_all_reduce(
                        dar, dacc[:, 0, :], 128, bass_isa.ReduceOp.add
                    )

                    def tail():
                        rb = rbp.tile([128, 512], F32, tag="rb")
                        nc.vector.reciprocal_approx_fast(rb, dar)
                        nc.vector.tensor_mul(ctx_sb[:, h, sc, :], cps, rb)
                else:
                    # final head: the whole tail chain is on the kernel's
                    # critical path (the last outproj groups wait on it).
                    # Split it into s-halves so the first half's gpsimd
                    # reduce+recip+mul finish ~1.8us earlier and the first
                    # tail outproj groups (so4=0,1) start sooner.
                    halves = []
                    for hf in range(2):
                        ssl = slice(256 * hf, 256 * hf + 256)
                        nc.vector.tensor_add(
                            dacc[:, 0, ssl], dacc[:, 0, ssl], dacc[:, 1, ssl]
                        )
                        dar = darp.tile([128, 256], F32, tag="dar")
                        nc.gpsimd.partition_all_reduce(
                            dar, dacc[:, 0, ssl], 128, bass_isa.ReduceOp.add
                        )
                        halves.append((ssl, dar))

                    def tail():
                        for hf, (ssl, dar) in enumerate(halves):
                            rb = rbp.tile([128, 256], F32, tag="rb")
                            nc.vector.reciprocal_approx_fast(rb, dar)
                            nc.vector.tensor_mul(
                                ctx_sb[:, h, sc, ssl], cps[:, ssl], rb
                            )
                return tail, sp0_next

            # one outproj row-block (128 out rows x full EMB) of sc,
            # as a generator yielding after each matmul
            def outproj_stream(sc, so4):
                ot4 = outs.tile([128, SC4, 512], BF, tag="ot")
                for ec in range(SC4):
                    ops = oacc.tile([128, 512], F32, tag="oacc")
                    for hl in range(HQ):
                        nc.tensor.matmul(
                            ops,
                            ctx_sb[:, hl, sc, so4 * 128:(so4 + 1) * 128],
                            wo_sb[:, hl, ec, :],
                            start=(hl == 0), stop=(hl == HQ - 1),
                        )
                        if hl == HQ - 1:
                            # copy split 2 DVE / 2 ACT: alternating engines
                            # decouples consecutive psum-bank releases, so a
                            # lagging DVE FIFO can't stall the ec+2 matmuls
                            if ec % 2 == 1:
                                nc.scalar.copy(ot4[:, ec, :], ops)
                            else:
                                nc.vector.tensor_copy(ot4[:, ec, :], ops)
                            if ec == SC4 - 1:
                                so = sc * 4 + so4
                                nc.sync.dma_start(
                                    out=out[so * 128:(so + 1) * 128, :].rearrange(
                                        "p (ec e) -> p ec e", e=512
                                    ),
                                    in_=ot4,
                                )
                        yield

            def chain_streams(items):
                for sc, so4 in items:
                    yield from outproj_stream(sc, so4)

            # outproj trails attention by one sc block plus one head-step
            # (so group (sc,0)'s hl=3 matmul is always emitted after the
            # deferred mul that writes ctx_sb[sc, h3])
            groups = [(sc, so4) for sc in range(SC4) for so4 in range(4)]
            filler = chain_streams(groups)
            empty = iter(())
            seq = [(sc, h) for sc in range(SC4) for h in range(HQ)]
            prev_tail = None
            sp0 = None
            for si, (sc, h) in enumerate(seq):
                nxt = seq[si + 1] if si + 1 < len(seq) else None
                prev_tail, sp0 = attn_head(
                    sc, h, empty if si < 5 else filler, prev_tail, sp0, nxt
                )
            prev_tail()
            for _ in filler:
                pass

        if loop_n is not None:
            with tc.For_i(0, loop_n, 1):
                _phases()
        else:
            _phases()

    nc.compile()
    return nc


def _get_nc():
    global _NC
    if _NC is None:
        _NC = _build_program()
    return _NC


def _rope_tables():
    half = HD // 2
    inv_freq = 1.0 / (10000.0 ** (np.arange(half, dtype=np.float64) * 2.0 / HD))
    ang = np.arange(S, dtype=np.float64)[:, None] * inv_freq[None, :]  # (S, 64)
    cos = np.concatenate([np.cos(ang), np.cos(ang)], axis=1).T  # (128, S)
    sin = np.concatenate([-np.sin(ang), np.sin(ang)], axis=1).T  # pre-signed
    return (np.ascontiguousarray(cos).astype(bfnp),
            np.ascontiguousarray(sin).astype(bfnp))


def build_in_maps(x, W_Q, W_K, W_V, W_O):
    x = np.asarray(x, dtype=np.float32)
    W_Q = np.asarray(W_Q, dtype=np.float32)
    W_K = np.asarray(W_K, dtype=np.float32)
    W_V = np.asarray(W_V, dtype=np.float32)
    W_O = np.asarray(W_O, dtype=np.float32)
    cos, sin = _rope_tables()
    in_maps = []
    xTb = [np.ascontiguousarray(x[b].T).astype(bfnp) for b in range(B)]
    for b in range(B):
        for t in range(TP):
            qheads = list(range(HQ * t, HQ * t + HQ))
            kvheads = [HKV * t + i for i in range(HKV)]
            idxq = [d * HEADS + h for h in qheads for d in range(HD)]
            idxkv = [d * KV + kv for kv in kvheads for d in range(HD)]
            rows_o = [h * HD + d for h in qheads for d in range(HD)]
            in_maps.append(dict(
                xT=xTb[b],
                wq=np.ascontiguousarray(W_Q[idxq, :].T).astype(bfnp),
                wk=np.ascontiguousarray(W_K[idxkv, :].T).astype(bfnp),
                wv=np.ascontiguousarray(W_V[idxkv, :].T).astype(bfnp),
                wo=np.ascontiguousarray(W_O[:, rows_o].T).astype(bfnp),
                cosT=cos,
                sinT=sin,
            ))
    return in_maps


def emulate_core(m):
    """Numpy emulation of the device math for one core's in_map."""
    xT = np.asarray(m["xT"], np.float32)      # (E, S)
    wq = np.asarray(m["wq"], np.float32)      # (E, 512)
    wk = np.asarray(m["wk"], np.float32)
    wv = np.asarray(m["wv"], np.float32)
    wo = np.asarray(m["wo"], np.float32)      # (512, E)
    cos = np.asarray(m["cosT"], np.float32)   # (128, S)
    sin = np.asarray(m["sinT"], np.float32)

    def bfq(a):
        return a.astype(bfnp).astype(np.float32)

    qT = bfq(wq.T @ xT)                       # (512, S)
    kT = bfq(wk.T @ xT)
    vT = bfq(wv.T @ xT)

    def rope(blkT):  # (128, S)
        xw = np.concatenate([blkT[64:], blkT[:64]], axis=0)
        return bfq(blkT * cos + xw * sin)

    ctxs = []
    for h in range(HQ):
        qh = rope(qT[h * 128:(h + 1) * 128])
        kvl = h // 2
        kh = rope(kT[kvl * 128:(kvl + 1) * 128])
        vh = vT[kvl * 128:(kvl + 1) * 128]
        scoresT = kh.T @ qh * SCALE           # (t, s)
        w = bfq(np.exp(scoresT))
        den = w.sum(axis=0)
        ctxT = bfq((vh @ w) / den[None, :])
        ctxs.append(ctxT)
    ctx = np.concatenate(ctxs, axis=0)        # (512, S)
    return bfq(ctx.T @ wo)


def combine_outs(outs):
    out = np.empty((B, S, EMB), dtype=np.float32)
    for b in range(B):
        acc = np.asarray(outs[TP * b]).astype(np.float32)
        for t in range(1, TP):
            acc = acc + np.asarray(outs[TP * b + t]).astype(np.float32)
        out[b] = acc
    return out


LAST_RESULTS = None


def kernel(x, W_Q, W_K, W_V, W_O):
    global LAST_RESULTS
    from concourse.bass_utils import run_bass_kernel_spmd

    nc = _get_nc()
    in_maps = build_in_maps(x, W_Q, W_K, W_V, W_O)
    res = run_bass_kernel_spmd(nc, in_maps, list(range(NCORES)))
    LAST_RESULTS = res
    outs = [r["out"] for r in res.results]
    return combine_outs(outs)

